# revision 38
# baseline (speedup 1.0000x reference)
"""Trainium2 Bass kernel for nn_BilinearDecoder: bilinear logits + diag mask +
bernoulli sampling + entropy, data-parallel over batch on 8 NeuronCores.

Math per batch b (reference):
    logits = E_b @ W @ E_b^T + l            [L, L]
    masked = logits - 1e8 * eye(L)
    p      = sigmoid(masked)
    samples= bernoulli(key(42), p)          == (u < p) == (masked > logit(u))
    entropy= p*softplus(-masked) + (1-p)*softplus(masked)
           = ent(a),  a = |masked|  (even function)

Entropy device formula: ent(m) is even in m and is approximated (rel-to-norm
err 3.9e-3 measured end-to-end, gate 2e-2) by a single even tanh pair read
DIRECTLY off masked (no |m| op needed):

    ent(m) = C * (tanh(B + D*m) + tanh(B - D*m))

Both evaluations use the Tanh table, so the scalar engine runs 2 table
passes per element with ONE table load for the whole kernel (the original
Exp/Ln/Silu scheme needed 3 passes and 4 table-set switches).  The pair
decays to exactly 0 at +-inf, so the diagonal (-inf after the unclamped
f16 drain) needs no special casing.  The device stores
stored = tanh(..) + tanh(..) in f16; the host applies the scale C (same
class of host postprocessing as a u8/fp8 dequant).

Engine balance per 512-row block (8 blocks per core), from measured rates
(DVE: f32-PSUM drain 1.27us/[128,1024], f16 TT 2.28us/[128,4,1024];
ACT 1 elem/cycle: 1.1us drain, 3.7us tanh; GPSIMD elementwise is 4x slower
AND degrades concurrent DVE ops 2-4x via SBUF-port contention - unused):
    DVE      3 of 4 drains (+l, clamp at -65300), samples = (masked > t)
             in place over the t tile (f16), stored = t1 + t2, kb copy
    ACT      4th drain via Identity+l (no clamp: diag underflows to -inf,
             safe through is_gt and the tanh pair - probed - and the host
             rewrites the diagonal), t1/t2 tanh pair
    ~9.0us(DVE) vs ~8.5us(ACT) per block, fully pipelined; PE runs 3
    [128,1024] PSUM tiles ahead; the last block quarters its tanh chain to
    shorten the dependency tail; stores ride the gpsimd queue, loads sync.
t = logit(u) fp16 precomputed host-side (fixed key(42) constant).
HBM traffic per core: 35.7 MB; DMA is bursty (peaks >400 GB/s), not the
binding constraint - engine time is, so all streams stay f16.
Host: masked diagonal overwritten with -1e8 (true value -1e8 + O(60), rel
err <= 6e-7); entropy diagonal zeroed (true value softplus(-1e8) ~ 0);
outputs upcast to f32.
"""
import sys
import json

sys.path.insert(0, '/opt/trn_rl_repo')

import numpy as np
import concourse.bass as bass
import concourse.tile as tile
from concourse import mybir
from concourse.masks import make_identity
from concourse.bass_utils import run_bass_kernel_spmd

# Problem shapes (hardcoded per contest rules)
B, L, H = 32, 1024, 128
N_CORES = 8
BPC = B // N_CORES           # batches per core
NCHUNK = L // 128            # 128-row chunks per batch
NEG_BIG = 1.0e8

F32 = mybir.dt.float32
F32R = mybir.dt.float32r
F16 = mybir.dt.float16
U16 = mybir.dt.uint16
U8 = mybir.dt.uint8

# entropy fit: ent(m) = ENT_C*(tanh(ENT_B + ENT_D*m) + tanh(ENT_B - ENT_D*m))
# (even tanh pair reads masked directly - no |m| op; decays to exactly 0 at
# +-inf so the diagonal needs no special casing on device; total entropy
# rel-err ~5e-3 including f16 effects, gate is 2e-2)
ENT_C = 1.6
ENT_B = 0.21932837388343565
ENT_D = 0.42714839706823937
# u8 quantization of stored = tanh+tanh in [0, 2*tanh(ENT_B)=0.4335]:
# scale chosen so the max (plus table-error slop) lands at 254.5 < 255;
# negative slop clamps to 0 in the u8 convert (probed).
ENT_QA = 584.4


def _split_waits_bir(d, limit=1):
    """This container's walrus accepts only `limit` sync-wait commands per
    instruction; Tile's kernel-tail drain carries several.  Move extras onto
    preceding Drain carriers on the same engine (order-preserving, safe)."""
    n = 0
    for fn in d['functions']:
        for bb in fn['blocks']:
            new_ins = []
            for ins in bb.get('instructions', []):
                si = ins.get('sync_info') or {}
                ow = si.get('on_wait') or []
                if len(ow) > limit:
                    extra = ow[:-limit]
                    si['on_wait'] = ow[-limit:]
                    for w in extra:
                        n += 1
                        new_ins.append({
                            "debug": ins.get("debug", 0),
                            "engine": ins["engine"],
                            "ins": [], "outs": [],
                            "is_reset_sema": False,
                            "name": f"{ins['name']}-wsplit{n}",
                            "opcode": "NoOp",
                            "sync_info": {"on_update": [], "on_wait": [w]},
                        })
                new_ins.append(ins)
            bb['instructions'] = new_ins
    return n


class PatchedBass(bass.Bass):
    def to_json_bytes(self):
        d = json.loads(super().to_json_bytes())
        _split_waits_bir(d)
        return json.dumps(d).encode()


def _build_nc():
    nc = PatchedBass("TRN2")

    enc = nc.dram_tensor("enc", [BPC, H, L], F32R, kind="ExternalInput")
    w_in = nc.dram_tensor("w_in", [H, H], F32R, kind="ExternalInput")
    lbias = nc.dram_tensor("lbias", [1], F32, kind="ExternalInput")
    thr = nc.dram_tensor("thr", [BPC, NCHUNK // 4, 128, 4, L], F16,
                         kind="ExternalInput")

    samples_o = nc.dram_tensor("samples_o", [L, BPC, L], F16, kind="ExternalOutput")
    masked_o = nc.dram_tensor("masked_o", [L, BPC, L], F16, kind="ExternalOutput")
    entropy_o = nc.dram_tensor("entropy_o", [L, BPC, L], U8, kind="ExternalOutput")

    with tile.TileContext(nc) as tc:
        with (
            tc.tile_pool(name="consts", bufs=1) as consts,
            tc.tile_pool(name="prep_ps", bufs=1, space="PSUM") as prep_ps,
            tc.tile_pool(name="x_ps", bufs=3, space="PSUM") as x_ps,
            tc.tile_pool(name="etbuf", bufs=4) as etbuf,
            tc.tile_pool(name="kbuf", bufs=2) as kbuf,
            tc.tile_pool(name="tpool", bufs=8) as tpool,
            tc.tile_pool(name="mpool", bufs=4) as mpool,
            tc.tile_pool(name="apool", bufs=4) as apool,
            tc.tile_pool(name="vpool", bufs=4) as vpool,
            tc.tile_pool(name="qpool", bufs=2) as qpool,
        ):
            # ---- input loads on the sync queue (the gpsimd DMA path is
            # software-dynamic and much slower per byte) ----
            wt = consts.tile([128, 128], F32R)
            nc.sync.dma_start(out=wt[:], in_=w_in[:, :])

            # ---- constants off the DVE critical path: identity and
            # -1e8*eye built on GPSIMD, activation biases tiny on DVE ----
            ident = consts.tile([128, 128], F32)
            make_identity(nc, ident[:])
            neg_eye = consts.tile([128, 128], F32)
            nc.gpsimd.memset(neg_eye[:], -NEG_BIG)
            nc.gpsimd.affine_select(
                out=neg_eye[:], in_=neg_eye[:],
                compare_op=mybir.AluOpType.is_equal,
                fill=0.0, base=0,
                pattern=[[-1, 128]], channel_multiplier=1,
            )
            # l broadcast to [128, 1] (per-partition bias operand)
            l_bc = consts.tile([128, 1], F32)
            l_bcast_ap = bass.AP(tensor=lbias, offset=0, ap=[[0, 128], [1, 1]])
            nc.gpsimd.dma_start(out=l_bc[:], in_=l_bcast_ap)
            # activation bias constant
            bt = consts.tile([128, 1], F32)
            nc.vector.memset(bt[:], ENT_B)

            # ---- ALL input loads issued upfront on the sync queue (SBUF
            # holds every et and t4 tile: ~162KB/partition total).  The
            # sync queue is therefore drained of loads ~halfway through the
            # run, so output stores can ride it later without EVER delaying
            # a load -- two store queues drain the write backlog in
            # parallel instead of one ----
            et_tiles = []
            for b in range(BPC):
                # E_b^T pre-transposed from host; split in half so the
                # first prep matmul starts as soon as half the load lands
                et = etbuf.tile([128, L], F32R)
                for half in range(2):
                    sl = slice(half * 512, (half + 1) * 512)
                    nc.sync.dma_start(out=et[:, sl], in_=enc[b][:, sl])
                et_tiles.append(et)
            t4_tiles = {}
            for b in range(BPC):
                for c4 in range(2):
                    t4 = tpool.tile([128, 4, L], F16)
                    nc.sync.dma_start(out=t4[:], in_=thr[b, c4])
                    t4_tiles[(b, c4)] = t4

            for b in range(BPC):
                et = et_tiles[b]
                ps_prep = prep_ps.tile([128, 1024], F32, tag="prep")
                for half in range(2):
                    sl = slice(half * 512, (half + 1) * 512)
                    nc.tensor.matmul(ps_prep[:, sl], wt[:],
                                     et[:, sl], start=True, stop=True)
                kb = kbuf.tile([128, L], F32R)
                nc.vector.tensor_copy(kb[:], ps_prep[:, :])

                for c4 in range(2):
                    t4 = t4_tiles[(b, c4)]
                    rows4 = slice(c4 * 512, (c4 + 1) * 512)
                    masked_t = mpool.tile([128, 4, L], F16)

                    for j in range(4):
                        # ---- x = E W E^T - 1e8*eye  (PSUM, f32r); one
                        # [128,1024] PSUM tile (2 banks) per 128-row chunk so
                        # PE can run up to 3 chunks ahead of the drains ----
                        c = 4 * c4 + j
                        rows = slice(c * 128, (c + 1) * 128)
                        ps_x = x_ps.tile([128, 1024], F32, tag="x")
                        for half in range(2):
                            sl = slice(half * 512, (half + 1) * 512)
                            diag_here = (c * 128 >= sl.start) and (c * 128 < sl.stop)
                            nc.tensor.matmul(
                                ps_x[:, sl], et[:, rows], kb[:, sl],
                                start=True, stop=not diag_here,
                            )
                        nc.tensor.matmul(
                            ps_x[:, rows], neg_eye[:], ident[:],
                            start=False, stop=True,
                        )

                        # ---- masked = x + l (f16).  3 drains per block on
                        # DVE (with clamp at -65300), the last on ACT via
                        # Identity (no clamp: diag underflows to -inf, which
                        # is safe through is_gt and the tanh pair, and the
                        # host rewrites the diagonal).  Balances DVE ~9.0us
                        # vs ACT ~8.4us per block ----
                        if j == 3:
                            nc.scalar.activation(
                                masked_t[:, j, :], ps_x[:],
                                mybir.ActivationFunctionType.Identity,
                                bias=l_bc[:, 0:1], scale=1.0,
                            )
                        else:
                            nc.vector.tensor_scalar(
                                masked_t[:, j, :], ps_x[:],
                                l_bc[:, 0:1], -65300.0,
                                op0=mybir.AluOpType.add, op1=mybir.AluOpType.max,
                            )

                    last_blk = (b == BPC - 1) and (c4 == 1)

                    a_t = apool.tile([128, 4, L], F16)
                    v2_t = vpool.tile([128, 4, L], F16)

                    def ent_chain(view):
                        """stored = tanh(B + D*masked) + tanh(B - D*masked),
                        both tanhs straight off masked (one table set, no
                        abs); host applies ent = C*stored."""
                        av, mv, vv = a_t[:, view, :], masked_t[:, view, :], v2_t[:, view, :]
                        nc.scalar.activation(
                            vv, mv, mybir.ActivationFunctionType.Tanh,
                            scale=ENT_D, bias=bt[:, 0:1],
                        )
                        nc.scalar.activation(
                            av, mv, mybir.ActivationFunctionType.Tanh,
                            scale=-ENT_D, bias=bt[:, 0:1],
                        )
                        nc.vector.tensor_tensor(
                            vv, vv, av, op=mybir.AluOpType.add,
                        )

                    if last_blk:
                        # quarter the chain so the final tanh->tanh->add
                        # dependency tail is ~4x shorter
                        ent_chain(slice(0, 1))
                        ent_chain(slice(1, 2))

                    # ---- samples = (masked > t), f16 in place over t4 ----
                    nc.vector.tensor_tensor(
                        t4[:], masked_t[:], t4[:],
                        op=mybir.AluOpType.is_gt,
                    )

                    if last_blk:
                        ent_chain(slice(2, 3))
                        ent_chain(slice(3, 4))
                    else:
                        ent_chain(slice(0, 4))

                    # ---- stores: masked+samples on gpsimd, ent on sync.
                    # All sync-queue load descriptors were issued upfront,
                    # so ent stores queue strictly behind them and never
                    # delay a load; the final write backlog drains on two
                    # queues in parallel.  Store-descriptor issues are
                    # emitted BEFORE the ent quant so the gpsimd engine
                    # doesn't hold them while waiting on the DVE add ----
                    nc.gpsimd.dma_start(
                        out=masked_o[rows4, b, :].rearrange("(t p) l -> p t l", p=128),
                        in_=masked_t[:],
                    )
                    nc.gpsimd.dma_start(
                        out=samples_o[rows4, b, :].rearrange("(t p) l -> p t l", p=128),
                        in_=t4[:],
                    )
                    # ent u8 quant on GPSIMD (2-stream TS, measured 4.5us
                    # clean, unlike the contention-heavy 3-stream TTs)
                    q_t = qpool.tile([128, 4, L], U8)
                    nc.gpsimd.tensor_scalar_mul(q_t[:], v2_t[:], ENT_QA)
                    nc.sync.dma_start(
                        out=entropy_o[rows4, b, :].rearrange("(t p) l -> p t l", p=128),
                        in_=q_t[:],
                    )

    return nc


_NC = None
_THR = None


def _get_nc():
    global _NC
    if _NC is None:
        _NC = _build_nc()
    return _NC


def _get_thr():
    """t = logit(u) with u = the exact uniforms jax.random.bernoulli(key(42))
    draws inside the reference.  Input-independent => precomputed constant.
    fp16 threshold rounding flips ~300 of 33.5M samples (norm gate allows
    ~6700).  u == 0 gives t = -inf; clamp to -65000 so the fp16 diag
    (clamped to ~-65300, or -inf on the ACT-drained blocks) still compares
    below it."""
    global _THR
    if _THR is None:
        import jax
        cpu = jax.devices("cpu")[0]
        with jax.default_device(cpu):
            u = np.asarray(
                jax.random.uniform(
                    jax.random.key(42), (L, B, L), dtype=np.float32
                )
            )
        u64 = u.astype(np.float64)
        with np.errstate(divide="ignore"):
            t = np.log(u64) - np.log1p(-u64)
        t = np.clip(t, -65000.0, 65000.0)
        _THR = t.astype(np.float16)
    return _THR


def _shard_inputs(encoder_output, W, l):
    """Build the per-core input maps (also used by test.py)."""
    encoder_output = np.ascontiguousarray(encoder_output, dtype=np.float32)
    W = np.ascontiguousarray(W, dtype=np.float32)
    l = np.ascontiguousarray(l, dtype=np.float32)
    thr = _get_thr()
    in_maps = []
    for i in range(N_CORES):
        bs = slice(i * BPC, (i + 1) * BPC)
        shard = thr[:, bs, :]
        # [L, BPC, L] -> [BPC, L/512, 128, 4, L]: row l = s*512 + t*128 + p
        tiled = np.ascontiguousarray(
            shard.reshape(NCHUNK // 4, 4, 128, BPC, L)
            .transpose(3, 0, 2, 1, 4)
        )
        in_maps.append({
            "enc": np.ascontiguousarray(encoder_output[bs].transpose(0, 2, 1)),
            "w_in": np.ascontiguousarray(W.T),
            "lbias": l,
            "thr": tiled,
        })
    return in_maps


def _unshard(results):
    samples = np.concatenate(
        [np.asarray(r["samples_o"]).astype(np.float32) for r in results], axis=1)
    masked = np.concatenate(
        [np.asarray(r["masked_o"]).astype(np.float32) for r in results], axis=1)
    stored = np.concatenate(
        [np.asarray(r["entropy_o"]).astype(np.float32) for r in results], axis=1)
    entropy = (ENT_C / ENT_QA) * stored
    # fp16 clamps the diagonal (-1e8 -> -65300 or -inf); true value is
    # -1e8 + logits_ii + l = -1e8 * (1 + O(6e-7)): write the constant.
    # entropy diagonal: p = sigmoid(-1e8) = 0 exactly -> ent = 0.
    ar = np.arange(L)
    masked[ar, :, ar] = -NEG_BIG
    entropy[ar, :, ar] = 0.0
    return samples, masked, entropy


def kernel(encoder_output, W, l):
    in_maps = _shard_inputs(encoder_output, W, l)
    nc = _get_nc()
    res = run_bass_kernel_spmd(nc, in_maps, core_ids=list(range(N_CORES)))
    return _unshard(res.results)


# revision 43
# speedup vs baseline: 4.7667x; 4.7667x over previous
"""Trainium2 Bass kernel for nn_BilinearDecoder: bilinear logits + diag mask +
bernoulli sampling + entropy, data-parallel over batch on 8 NeuronCores.

Math per batch b (reference):
    logits = E_b @ W @ E_b^T + l            [L, L]
    masked = logits - 1e8 * eye(L)
    p      = sigmoid(masked)
    samples= bernoulli(key(42), p)          == (u < p) == (masked > logit(u))
    entropy= p*softplus(-masked) + (1-p)*softplus(masked)
           = ent(a),  a = |masked|  (even function)

Entropy device formula: ent(m) is even in m and is approximated (rel-to-norm
err 3.9e-3 measured end-to-end, gate 2e-2) by a single even tanh pair read
DIRECTLY off masked (no |m| op needed):

    ent(m) = C * (tanh(B + D*m) + tanh(B - D*m))

Both evaluations use the Tanh table, so the scalar engine runs 2 table
passes per element with ONE table load for the whole kernel (the original
Exp/Ln/Silu scheme needed 3 passes and 4 table-set switches).  The pair
decays to exactly 0 at +-inf, so the diagonal (-inf after the unclamped
f16 drain) needs no special casing.  The device stores
stored = tanh(..) + tanh(..) in f16; the host applies the scale C (same
class of host postprocessing as a u8/fp8 dequant).

Engine balance per 512-row block (8 blocks per core), from measured rates
(DVE: f32-PSUM drain 1.27us/[128,1024], f16 TT 2.28us/[128,4,1024];
ACT 1 elem/cycle: 1.1us drain, 3.7us tanh; GPSIMD elementwise is 4x slower
AND degrades concurrent DVE ops 2-4x via SBUF-port contention - unused):
    DVE      3 of 4 drains (+l, clamp at -65300), samples = (masked > t)
             in place over the t tile (f16), stored = t1 + t2, kb copy
    ACT      4th drain via Identity+l (no clamp: diag underflows to -inf,
             safe through is_gt and the tanh pair - probed - and the host
             rewrites the diagonal), t1/t2 tanh pair
    ~9.0us(DVE) vs ~8.5us(ACT) per block, fully pipelined; PE runs 3
    [128,1024] PSUM tiles ahead; the last block quarters its tanh chain to
    shorten the dependency tail; stores ride the gpsimd queue, loads sync.
t = logit(u) fp16 precomputed host-side (fixed key(42) constant).
HBM traffic per core: 35.7 MB; DMA is bursty (peaks >400 GB/s), not the
binding constraint - engine time is, so all streams stay f16.
Host: masked diagonal overwritten with -1e8 (true value -1e8 + O(60), rel
err <= 6e-7); entropy diagonal zeroed (true value softplus(-1e8) ~ 0);
outputs upcast to f32.
"""
import sys
import json

sys.path.insert(0, '/opt/trn_rl_repo')

import numpy as np
import concourse.bass as bass
import concourse.tile as tile
from concourse import mybir
from concourse.masks import make_identity
from concourse.bass_utils import run_bass_kernel_spmd

# Problem shapes (hardcoded per contest rules)
B, L, H = 32, 1024, 128
N_CORES = 8
BPC = B // N_CORES           # batches per core
NCHUNK = L // 128            # 128-row chunks per batch
NEG_BIG = 1.0e8

F32 = mybir.dt.float32
F32R = mybir.dt.float32r
F16 = mybir.dt.float16
U16 = mybir.dt.uint16
U8 = mybir.dt.uint8

# entropy fit: ent(m) = ENT_C*(tanh(ENT_B + ENT_D*m) + tanh(ENT_B - ENT_D*m))
# (even tanh pair reads masked directly - no |m| op; decays to exactly 0 at
# +-inf so the diagonal needs no special casing on device; total entropy
# rel-err ~5e-3 including f16 effects, gate is 2e-2)
ENT_C = 1.6
ENT_B = 0.21932837388343565
ENT_D = 0.42714839706823937


def _split_waits_bir(d, limit=1):
    """This container's walrus accepts only `limit` sync-wait commands per
    instruction; Tile's kernel-tail drain carries several.  Move extras onto
    preceding Drain carriers on the same engine (order-preserving, safe)."""
    n = 0
    for fn in d['functions']:
        for bb in fn['blocks']:
            new_ins = []
            for ins in bb.get('instructions', []):
                si = ins.get('sync_info') or {}
                ow = si.get('on_wait') or []
                if len(ow) > limit:
                    extra = ow[:-limit]
                    si['on_wait'] = ow[-limit:]
                    for w in extra:
                        n += 1
                        new_ins.append({
                            "debug": ins.get("debug", 0),
                            "engine": ins["engine"],
                            "ins": [], "outs": [],
                            "is_reset_sema": False,
                            "name": f"{ins['name']}-wsplit{n}",
                            "opcode": "NoOp",
                            "sync_info": {"on_update": [], "on_wait": [w]},
                        })
                new_ins.append(ins)
            bb['instructions'] = new_ins
    return n


class PatchedBass(bass.Bass):
    def to_json_bytes(self):
        d = json.loads(super().to_json_bytes())
        _split_waits_bir(d)
        return json.dumps(d).encode()


def _build_nc():
    nc = PatchedBass("TRN2")

    enc = nc.dram_tensor("enc", [BPC, H, L], F32R, kind="ExternalInput")
    w_in = nc.dram_tensor("w_in", [H, H], F32R, kind="ExternalInput")
    lbias = nc.dram_tensor("lbias", [1], F32, kind="ExternalInput")
    thr = nc.dram_tensor("thr", [BPC, NCHUNK // 4, 128, 4, L], F16,
                         kind="ExternalInput")

    samples_o = nc.dram_tensor("samples_o", [L, BPC, L], F16, kind="ExternalOutput")
    masked_o = nc.dram_tensor("masked_o", [L, BPC, L], F16, kind="ExternalOutput")
    entropy_o = nc.dram_tensor("entropy_o", [L, BPC, L], F16, kind="ExternalOutput")

    with tile.TileContext(nc) as tc:
        with (
            tc.tile_pool(name="consts", bufs=1) as consts,
            tc.tile_pool(name="prep_ps", bufs=1, space="PSUM") as prep_ps,
            tc.tile_pool(name="x_ps", bufs=3, space="PSUM") as x_ps,
            tc.tile_pool(name="etbuf", bufs=4) as etbuf,
            tc.tile_pool(name="kbuf", bufs=2) as kbuf,
            tc.tile_pool(name="tpool", bufs=8) as tpool,
            tc.tile_pool(name="mpool", bufs=4) as mpool,
            tc.tile_pool(name="apool", bufs=4) as apool,
            tc.tile_pool(name="vpool", bufs=4) as vpool,
        ):
            # ---- input loads on the sync queue (the gpsimd DMA path is
            # software-dynamic and much slower per byte) ----
            wt = consts.tile([128, 128], F32R)
            nc.sync.dma_start(out=wt[:], in_=w_in[:, :])

            # ---- constants off the DVE critical path: identity and
            # -1e8*eye built on GPSIMD, activation biases tiny on DVE ----
            ident = consts.tile([128, 128], F32)
            make_identity(nc, ident[:])
            neg_eye = consts.tile([128, 128], F32)
            nc.gpsimd.memset(neg_eye[:], -NEG_BIG)
            nc.gpsimd.affine_select(
                out=neg_eye[:], in_=neg_eye[:],
                compare_op=mybir.AluOpType.is_equal,
                fill=0.0, base=0,
                pattern=[[-1, 128]], channel_multiplier=1,
            )
            # l broadcast to [128, 1] (per-partition bias operand)
            l_bc = consts.tile([128, 1], F32)
            l_bcast_ap = bass.AP(tensor=lbias, offset=0, ap=[[0, 128], [1, 1]])
            nc.gpsimd.dma_start(out=l_bc[:], in_=l_bcast_ap)
            # activation bias constant
            bt = consts.tile([128, 1], F32)
            nc.vector.memset(bt[:], ENT_B)

            # ---- ALL input loads issued upfront on the sync queue (SBUF
            # holds every et and t4 tile: ~162KB/partition total).  The
            # sync queue is therefore drained of loads ~halfway through the
            # run, so output stores can ride it later without EVER delaying
            # a load -- two store queues drain the write backlog in
            # parallel instead of one ----
            et_tiles = []
            for b in range(BPC):
                # E_b^T pre-transposed from host; split in half so the
                # first prep matmul starts as soon as half the load lands
                et = etbuf.tile([128, L], F32R)
                for half in range(2):
                    sl = slice(half * 512, (half + 1) * 512)
                    nc.sync.dma_start(out=et[:, sl], in_=enc[b][:, sl])
                et_tiles.append(et)
            t4_tiles = {}
            for b in range(BPC):
                for c4 in range(2):
                    t4 = tpool.tile([128, 4, L], F16)
                    nc.sync.dma_start(out=t4[:], in_=thr[b, c4])
                    t4_tiles[(b, c4)] = t4

            for b in range(BPC):
                et = et_tiles[b]
                ps_prep = prep_ps.tile([128, 1024], F32, tag="prep")
                for half in range(2):
                    sl = slice(half * 512, (half + 1) * 512)
                    nc.tensor.matmul(ps_prep[:, sl], wt[:],
                                     et[:, sl], start=True, stop=True)
                kb = kbuf.tile([128, L], F32R)
                nc.vector.tensor_copy(kb[:], ps_prep[:, :])

                for c4 in range(2):
                    t4 = t4_tiles[(b, c4)]
                    rows4 = slice(c4 * 512, (c4 + 1) * 512)
                    masked_t = mpool.tile([128, 4, L], F16)

                    for j in range(4):
                        # ---- x = E W E^T - 1e8*eye  (PSUM, f32r); one
                        # [128,1024] PSUM tile (2 banks) per 128-row chunk so
                        # PE can run up to 3 chunks ahead of the drains ----
                        c = 4 * c4 + j
                        rows = slice(c * 128, (c + 1) * 128)
                        ps_x = x_ps.tile([128, 1024], F32, tag="x")
                        for half in range(2):
                            sl = slice(half * 512, (half + 1) * 512)
                            diag_here = (c * 128 >= sl.start) and (c * 128 < sl.stop)
                            nc.tensor.matmul(
                                ps_x[:, sl], et[:, rows], kb[:, sl],
                                start=True, stop=not diag_here,
                            )
                        nc.tensor.matmul(
                            ps_x[:, rows], neg_eye[:], ident[:],
                            start=False, stop=True,
                        )

                        # ---- masked = x + l (f16).  3 drains per block on
                        # DVE (with clamp at -65300), the last on ACT via
                        # Identity (no clamp: diag underflows to -inf, which
                        # is safe through is_gt and the tanh pair, and the
                        # host rewrites the diagonal).  Balances DVE ~9.0us
                        # vs ACT ~8.4us per block ----
                        if j == 3:
                            nc.scalar.activation(
                                masked_t[:, j, :], ps_x[:],
                                mybir.ActivationFunctionType.Identity,
                                bias=l_bc[:, 0:1], scale=1.0,
                            )
                        else:
                            nc.vector.tensor_scalar(
                                masked_t[:, j, :], ps_x[:],
                                l_bc[:, 0:1], -65300.0,
                                op0=mybir.AluOpType.add, op1=mybir.AluOpType.max,
                            )

                    last_blk = (b == BPC - 1) and (c4 == 1)

                    a_t = apool.tile([128, 4, L], F16)
                    v2_t = vpool.tile([128, 4, L], F16)

                    def ent_chain(view):
                        """stored = tanh(B + D*masked) + tanh(B - D*masked),
                        both tanhs straight off masked (one table set, no
                        abs); host applies ent = C*stored."""
                        av, mv, vv = a_t[:, view, :], masked_t[:, view, :], v2_t[:, view, :]
                        nc.scalar.activation(
                            vv, mv, mybir.ActivationFunctionType.Tanh,
                            scale=ENT_D, bias=bt[:, 0:1],
                        )
                        nc.scalar.activation(
                            av, mv, mybir.ActivationFunctionType.Tanh,
                            scale=-ENT_D, bias=bt[:, 0:1],
                        )
                        nc.vector.tensor_tensor(
                            vv, vv, av, op=mybir.AluOpType.add,
                        )

                    if last_blk:
                        # quarter the chain so the final tanh->tanh->add
                        # dependency tail is ~4x shorter
                        ent_chain(slice(0, 1))
                        ent_chain(slice(1, 2))

                    # ---- samples = (masked > t), f16 in place over t4 ----
                    nc.vector.tensor_tensor(
                        t4[:], masked_t[:], t4[:],
                        op=mybir.AluOpType.is_gt,
                    )

                    if last_blk:
                        ent_chain(slice(2, 3))
                        ent_chain(slice(3, 4))
                    else:
                        ent_chain(slice(0, 4))

                    # ---- stores: masked+samples on gpsimd, ent on sync.
                    # All sync-queue load descriptors were issued upfront,
                    # so ent stores queue strictly behind them and never
                    # delay a load; the final write backlog drains on two
                    # queues in parallel.  (u8-quantized entropy via a
                    # GPSIMD tensor_scalar was tried to cut the write
                    # bytes: 5x regression - any GPSIMD elementwise op in
                    # the steady state destroys the schedule) ----
                    nc.gpsimd.dma_start(
                        out=masked_o[rows4, b, :].rearrange("(t p) l -> p t l", p=128),
                        in_=masked_t[:],
                    )
                    nc.gpsimd.dma_start(
                        out=samples_o[rows4, b, :].rearrange("(t p) l -> p t l", p=128),
                        in_=t4[:],
                    )
                    nc.sync.dma_start(
                        out=entropy_o[rows4, b, :].rearrange("(t p) l -> p t l", p=128),
                        in_=v2_t[:],
                    )

    return nc


_NC = None
_THR = None


def _get_nc():
    global _NC
    if _NC is None:
        _NC = _build_nc()
    return _NC


def _get_thr():
    """t = logit(u) with u = the exact uniforms jax.random.bernoulli(key(42))
    draws inside the reference.  Input-independent => precomputed constant.
    fp16 threshold rounding flips ~300 of 33.5M samples (norm gate allows
    ~6700).  u == 0 gives t = -inf; clamp to -65000 so the fp16 diag
    (clamped to ~-65300, or -inf on the ACT-drained blocks) still compares
    below it."""
    global _THR
    if _THR is None:
        import jax
        cpu = jax.devices("cpu")[0]
        with jax.default_device(cpu):
            u = np.asarray(
                jax.random.uniform(
                    jax.random.key(42), (L, B, L), dtype=np.float32
                )
            )
        u64 = u.astype(np.float64)
        with np.errstate(divide="ignore"):
            t = np.log(u64) - np.log1p(-u64)
        t = np.clip(t, -65000.0, 65000.0)
        _THR = t.astype(np.float16)
    return _THR


def _shard_inputs(encoder_output, W, l):
    """Build the per-core input maps (also used by test.py)."""
    encoder_output = np.ascontiguousarray(encoder_output, dtype=np.float32)
    W = np.ascontiguousarray(W, dtype=np.float32)
    l = np.ascontiguousarray(l, dtype=np.float32)
    thr = _get_thr()
    in_maps = []
    for i in range(N_CORES):
        bs = slice(i * BPC, (i + 1) * BPC)
        shard = thr[:, bs, :]
        # [L, BPC, L] -> [BPC, L/512, 128, 4, L]: row l = s*512 + t*128 + p
        tiled = np.ascontiguousarray(
            shard.reshape(NCHUNK // 4, 4, 128, BPC, L)
            .transpose(3, 0, 2, 1, 4)
        )
        in_maps.append({
            "enc": np.ascontiguousarray(encoder_output[bs].transpose(0, 2, 1)),
            "w_in": np.ascontiguousarray(W.T),
            "lbias": l,
            "thr": tiled,
        })
    return in_maps


def _unshard(results):
    samples = np.concatenate(
        [np.asarray(r["samples_o"]).astype(np.float32) for r in results], axis=1)
    masked = np.concatenate(
        [np.asarray(r["masked_o"]).astype(np.float32) for r in results], axis=1)
    stored = np.concatenate(
        [np.asarray(r["entropy_o"]).astype(np.float32) for r in results], axis=1)
    entropy = ENT_C * stored
    # fp16 clamps the diagonal (-1e8 -> -65300 or -inf); true value is
    # -1e8 + logits_ii + l = -1e8 * (1 + O(6e-7)): write the constant.
    # entropy diagonal: p = sigmoid(-1e8) = 0 exactly -> ent = 0.
    ar = np.arange(L)
    masked[ar, :, ar] = -NEG_BIG
    entropy[ar, :, ar] = 0.0
    return samples, masked, entropy


def kernel(encoder_output, W, l):
    in_maps = _shard_inputs(encoder_output, W, l)
    nc = _get_nc()
    res = run_bass_kernel_spmd(nc, in_maps, core_ids=list(range(N_CORES)))
    return _unshard(res.results)
